# revision 24
# baseline (speedup 1.0000x reference)
"""Multi-head self-attention (b=4, n=2048, f=1024, h=16) on 8 trn2 NeuronCores.

Sharding: core c -> batch c//2, head-half c%2 (8 heads of 64 dims each).
Each core computes its 8 heads' attention and a partial output projection
(attn_slice @ Wo_rows); host sums the two partials per batch and adds bo.

v3 over the original kernel (all matmul operands bf16, PSUM fp32):
  - exp has NO bias: the additive mask term exp(m_j) is folded into the AV
    stationaries instead (s1 = e^{+m}[v|1], s0 = [v|1]). This frees the
    choice of AV stationary per query block.
  - the host permutation places tokens so query blocks 0/2/3 are pure
    (block 2's mask value mu arrives as data via a third stationary
    s2 = e^{mu m}[v|1]); only block 1 needs the dual-variant select.
    AV work: 10 matmuls per (pair, key-chunk) vs 12 before.
  - the exp stream on ScalarE (256 x [128,1024] chunks at ~1.15us each,
    ~295us total) is the critical path; PE work (~310us) runs just under
    it. Phase 1 is interleaved into the attention stream: only kT/qT
    chunk 0 and half of v run up front (~30us); the rest of v, kT/qT
    chunks 1-3, and the O-projection drain as micro-ops into the PE slack
    of the pure query blocks, so ScalarE starts exp'ing as early as
    possible and never waits.
"""

import sys

sys.path.insert(0, "/opt/trn_rl_repo")

import numpy as np
import ml_dtypes

import concourse.bass as bass
import concourse.bacc as bacc
import concourse.mybir as mybir
import concourse.tile as tile
from concourse import bass_utils

BF16 = mybir.dt.bfloat16
F32 = mybir.dt.float32
FP8 = mybir.dt.float8e4
NPBF16 = ml_dtypes.bfloat16
NPFP8 = ml_dtypes.float8_e4m3
DR = mybir.MatmulPerfMode.DoubleRow
MUL = mybir.AluOpType.mult
ADD = mybir.AluOpType.add

B, N, F, H, HD = 4, 2048, 1024, 16, 64
FH = 512          # features per core (8 heads)
NC_ = 8           # cores
NTOKC = N // 128  # 16 token chunks
NIBLK = N // 512  # 4 query blocks
NJ = N // 128     # 16 key chunks
NPAIR = 4         # head pairs per core
EXPFN = mybir.ActivationFunctionType.Exp
SROW = 66         # stationary cols per head
SJ = 8 * SROW     # stationary cols per key chunk (528)
NLEADV = 4        # v token-chunks computed up front; the rest drain
WS = 32.0         # q/k weight pre-scale (escapes fp8 subnormals)


def _ap3(t, off, s1, n1, s2, n2):
    """3D AP view [128, n1 (stride s1), n2 (stride s2)] at t+off."""
    return bass.AP(tensor=t.tensor, offset=t.offset + off,
                   ap=[t.ap[0], [s1, n1], [s2, n2]])


def _emit(nc, tc, d, sorted_mode):
    consts = tc.alloc_tile_pool(name="consts", bufs=1)
    persist = tc.alloc_tile_pool(name="persist", bufs=1)

    # ---- persistent activations ----------------------------------------
    qT_sb = persist.tile([128, 4 * N], BF16)   # [feat, tok], fhc at cols fhc*N
    kT_sb = persist.tile([128, 4 * N], BF16)
    s0_sb = persist.tile([128, NJ * SJ + 64], BF16)  # [j][h][66]: [vb|1]
    s1_sb = persist.tile([128, NJ * SJ + 64], BF16)  # e^{+m} * [vb|1]
    attnT = persist.tile([128, 4 * N], BF16)   # normalized attn, [feat, tok]

    # ================= phase 1 (lead-in part) ============================
    p1sb = tc.alloc_tile_pool(name="p1sb", bufs=1)
    pkt = tc.alloc_tile_pool(name="pkt", bufs=1, space="PSUM")

    bqk = consts.tile([128, 8], F32)       # bq chunks (0-3), bk (4-7)
    nc.sync.dma_start(out=bqk, in_=d["bqk"])
    nc.vector.memset(s0_sb[:], 0.0)
    nc.vector.memset(s1_sb[:], 0.0)
    # exp table warm-up (~2.7us) while DMAs run
    warm = consts.tile([128, 8], BF16)
    nc.scalar.activation(out=warm, in_=bqk, func=EXPFN, scale=0.0)

    x8_sb = p1sb.tile([128, 8 * N], FP8)
    wk_sb = p1sb.tile([128, 8 * FH], FP8)  # [g 4][pl 2][col 512], x32
    wq_sb = p1sb.tile([128, 8 * FH], FP8)
    nc.sync.dma_start(out=wk_sb, in_=d["wkp"])
    nc.sync.dma_start(out=wq_sb, in_=d["wqp"])
    for fc in range(8):
        nc.sync.dma_start(out=x8_sb[:, fc * N:(fc + 1) * N],
                          in_=d["x8"][fc * 128:(fc + 1) * 128, :])

    pools = {"proj": pkt}  # phase 2 rebinds this to its own PSUM pool

    def proj_qk_ops(w_sb, fhc, win, bias_col, out_sb, tagsel, grp=None):
        """Micro-ops for one [128,512] window of a q/k DR-fp8 projection:
        4 paired-contraction matmuls plus the scale+bias add."""
        st = {}
        ops = []
        grp = grp or f"f{fhc}"
        for g in range(4):
            def mm(g=g, st=st):
                if g == 0:
                    st["pk"] = pools["proj"].tile([128, 512], F32,
                                                  tag=tagsel, name="pk")
                lhsT = _ap3(w_sb, g * 1024 + fhc * 128, 512, 2, 1, 128)
                rhs = _ap3(x8_sb, (2 * g) * N + win * 512, N, 2, 1, 512)
                nc.tensor.matmul(st["pk"][:], lhsT, rhs, start=(g == 0),
                                 stop=(g == 3), perf_mode=DR)
            ops.append(("pe", grp, mm))

        def fin(st=st):
            nc.vector.tensor_scalar(
                out=out_sb[:, fhc * N + win * 512: fhc * N + win * 512 + 512],
                in0=st["pk"][:], scalar1=1.0 / WS,
                scalar2=bqk[:, bias_col:bias_col + 1], op0=MUL, op1=ADD)
        ops.append(("dve", grp, fin))
        return ops

    # up front: all of kT chunk 0, and qT chunk 0's first-block window
    if sorted_mode:
        for win in range(4):
            for _, _, op in proj_qk_ops(wk_sb, 0, win, 4, kT_sb,
                                        f"pp{win % 2}"):
                op()
        for _, _, op in proj_qk_ops(wq_sb, 0, 0, 0, qT_sb, "pp2"):
            op()
    else:
        for fhc in range(4):
            for win in range(4):
                for _, _, op in proj_qk_ops(wk_sb, fhc, win, 4 + fhc, kT_sb,
                                            f"pp{win % 2}"):
                    op()
                for _, _, op in proj_qk_ops(wq_sb, fhc, win, fhc, qT_sb,
                                            f"pp{2 + win % 2}"):
                    op()

    # loads for the v/attention part; x (bf16) arrives token-block-major so
    # v(tokc) starts after 1/4 of it
    xT_sb = p1sb.tile([128, 8 * N], BF16)
    wv_sb = p1sb.tile([128, 8 * FH], BF16)
    for fc in range(8):
        nc.sync.dma_start(out=wv_sb[:, fc * FH:(fc + 1) * FH],
                          in_=d["wv"][fc * 128:(fc + 1) * 128, :])
    for tb in range(4):
        for fc in range(8):
            nc.sync.dma_start(
                out=xT_sb[:, fc * N + tb * 512: fc * N + (tb + 1) * 512],
                in_=d["xT"][fc * 128:(fc + 1) * 128, tb * 512:(tb + 1) * 512])
    ep1 = consts.tile([128, NTOKC], F32)   # e^{+m}
    nc.sync.dma_start(out=ep1, in_=d["ep1"])
    mjb2 = consts.tile([128, NJ], F32)     # exp bias for block 2: mu*m_j
    nc.sync.dma_start(out=mjb2, in_=d["mjb2"])
    bvb = consts.tile([128, FH], F32)
    nc.sync.dma_start(out=bvb, in_=d["bvb"])
    nmr = 2 if sorted_mode else 4
    mr4 = consts.tile([nmr, N], F32)       # select rows [m, 1-m, (m, 1-m)]
    nc.sync.dma_start(out=mr4, in_=d["mr4"][0:nmr, :])
    wo_sb = consts.tile([128, 4 * 1024], BF16)
    for fc in range(4):
        nc.sync.dma_start(out=wo_sb[:, fc * 1024:(fc + 1) * 1024],
                          in_=d["wo"][fc * 128:(fc + 1) * 128, :])

    # v projection + stationaries; AV consumes s[j] at key chunk j
    vstg = p1sb.tile([128, FH], F32, tag="vstg")

    def v_ops(tokc, tagsel):
        st = {}
        ops = []
        grp = f"v{tokc}"
        for fc in range(8):
            def mm(fc=fc, tokc=tokc, st=st):
                if fc == 0:
                    st["pv"] = pools["proj"].tile([128, FH], F32,
                                                  tag=tagsel, name="pv")
                nc.tensor.matmul(
                    st["pv"][:],
                    xT_sb[:, fc * N + tokc * 128: fc * N + (tokc + 1) * 128],
                    wv_sb[:, fc * FH:(fc + 1) * FH],
                    start=(fc == 0), stop=(fc == 7))
            ops.append(("pe", grp, mm))

        def fin(tokc=tokc, st=st):
            nc.vector.tensor_add(out=vstg, in0=st["pv"][:], in1=bvb)
            base = tokc * SJ
            vv = vstg[:].rearrange("p (h c) -> p h c", h=8)
            for s_sb, scol in ((s0_sb, None), (s1_sb, ep1)):
                sv = s_sb[:, base:base + SJ].rearrange("p (h c) -> p h c", h=8)
                if scol is None:
                    nc.vector.tensor_copy(out=sv[:, :, 0:64], in_=vv)
                    nc.vector.memset(sv[:, :, 64:65], 1.0)
                else:
                    nc.vector.tensor_scalar_mul(
                        out=sv[:, :, 0:64], in0=vv,
                        scalar1=scol[:, tokc:tokc + 1])
                    colb = bass.AP(
                        tensor=scol.tensor,
                        offset=scol[:, tokc:tokc + 1].offset,
                        ap=[scol[:, tokc:tokc + 1].ap[0], [0, 8], [1, 1]])
                    nc.vector.tensor_copy(out=sv[:, :, 64:65], in_=colb)
        ops.append(("dve", grp, fin))
        return ops

    nleadv = NLEADV if sorted_mode else NTOKC
    for tokc in range(nleadv):
        for _, _, op in v_ops(tokc, f"pp{tokc % 2}"):
            op()

    # deferred work drains into attention slack (pure blocks)
    pending = []
    if sorted_mode:
        pending.extend(proj_qk_ops(wq_sb, 0, 2, 0, qT_sb, "acc3", grp="f0w2"))
        for tokc in range(NLEADV, 8):
            pending.extend(v_ops(tokc, "acc1" if tokc % 2 else "acc3"))
        for win in range(4):
            pending.extend(
                proj_qk_ops(wk_sb, 1, win, 5, kT_sb, "acc1"))
            pending.extend(
                proj_qk_ops(wq_sb, 1, win, 1, qT_sb, "acc3"))
        for tokc in range(8, NTOKC):
            pending.extend(v_ops(tokc, "acc1" if tokc % 2 else "acc3"))
        for fhc in (2, 3):
            for win in range(4):
                pending.extend(
                    proj_qk_ops(wk_sb, fhc, win, 4 + fhc, kT_sb, "acc1"))
                pending.extend(
                    proj_qk_ops(wq_sb, fhc, win, fhc, qT_sb, "acc3"))
        pending.extend(proj_qk_ops(wq_sb, 0, 1, 0, qT_sb, "acc3", grp="f0w1"))
        pending.extend(proj_qk_ops(wq_sb, 0, 3, 0, qT_sb, "acc3", grp="f0w3"))

    pkt.release()
    if not sorted_mode:
        p1sb.release()  # nothing defers in dual mode; free xT/w space

    def drain(npe):
        done = 0
        while pending and done < npe:
            kind, _, op = pending.pop(0)
            op()
            if kind == "pe":
                done += 1
        # trailing dve ops ride along for free
        while pending and pending[0][0] == "dve":
            pending.pop(0)[2]()

    def drain_group(grp):
        """Pop (in order) until no ops of group `grp` remain."""
        while any(g == grp for _, g, _ in pending):
            pending.pop(0)[2]()

    # ================= phase 2: attention ================================
    with tc.tile_pool(name="pP", bufs=2, space="PSUM") as pP, \
         tc.tile_pool(name="pacc", bufs=1, space="PSUM") as pacc, \
         tc.tile_pool(name="sexp", bufs=2) as sexp, \
         tc.tile_pool(name="episb", bufs=1) as episb, \
         tc.tile_pool(name="rblp", bufs=1) as rblp, \
         tc.tile_pool(name="osb", bufs=2) as osb, \
         tc.tile_pool(name="epidr", bufs=2, space="DRAM") as epidr:

        pools["proj"] = pacc

        def o_ops_for_iblk(ib, tags, fin_act=False):
            ops = []
            from itertools import cycle
            tagc = cycle(tags)
            for tokc in range(ib * 4, ib * 4 + 4):
                for half in range(2):
                    st = {}
                    for fc in range(4):
                        def mm(fc=fc, tokc=tokc, half=half, st=st):
                            if fc == 0:
                                st["po"] = pacc.tile(
                                    [128, 512], F32, tag=next(tagc), name="po")
                            nc.tensor.matmul(
                                st["po"][:],
                                attnT[:, fc * N + tokc * 128: fc * N + (tokc + 1) * 128],
                                wo_sb[:, fc * 1024 + half * 512: fc * 1024 + half * 512 + 512],
                                start=(fc == 0), stop=(fc == 3))
                        ops.append(("pe", f"o{ib}", mm))

                    def fin(tokc=tokc, half=half, st=st):
                        ot = osb.tile([128, 512], F32, tag="ot", name="ot")
                        if fin_act:
                            nc.scalar.activation(
                                out=ot, in_=st["po"][:],
                                func=mybir.ActivationFunctionType.Copy)
                        else:
                            nc.vector.tensor_copy(out=ot, in_=st["po"][:])
                        nc.sync.dma_start(
                            out=d["y"][tokc * 128:(tokc + 1) * 128,
                                       half * 512:(half + 1) * 512],
                            in_=ot)
                    ops.append(("dve", f"o{ib}", fin))
            return ops

        # global QK issue stream: stays 2 exp-steps ahead ACROSS pair and
        # block boundaries so the exp stream never sees a pipeline refill
        qkq = []          # [(key, closure)] not yet issued
        Pready = {}       # key -> psum logits tile

        def issue_qk():
            key, fn = qkq.pop(0)
            Pready[key] = fn()

        # pure blocks first for drain slack; the dual block third
        iblk_order = [0, 2, 1, 3] if sorted_mode else [0, 1, 2, 3]
        for iblk in iblk_order:
            if sorted_mode and iblk == 0:
                active, stats = [0, 2], {0: s1_sb, 2: s1_sb}
            elif sorted_mode and iblk == 2:
                active, stats = [0, 2], {0: s0_sb, 2: s0_sb}
            elif sorted_mode and iblk == 3:
                active, stats = [1, 3], {1: s0_sb, 3: s0_sb}
            else:
                active = [0, 1, 2, 3]
                stats = {0: s1_sb, 1: s0_sb, 2: s1_sb, 3: s0_sb}
            dual = len(active) == 4
            ndrain = 6 if (sorted_mode and iblk == 0) else 2
            first_blk = iblk == iblk_order[0]
            for pair in range(NPAIR):
                accs = {
                    v: pacc.tile([128, 512], F32, tag=f"acc{v}", name=f"acc{v}")
                    for v in active
                }
                def qk(j, pair=pair, iblk=iblk):
                    # deferred kT/qT chunks must be emitted before this
                    # pair's first QK reads them
                    if sorted_mode and j == 0 and pair > 0:
                        drain_group(f"f{pair}")
                    if sorted_mode and j == 0 and pair == 0 and iblk != 0:
                        drain_group(f"f0w{iblk}")
                    P = pP.tile([128, 1024], F32, tag="logits")
                    for hl, tp in ((0, 0), (1, 64)):
                        nc.tensor.matmul(
                            P[:, hl * 512:(hl + 1) * 512],
                            kT_sb[tp:tp + 64, pair * N + j * 128: pair * N + (j + 1) * 128],
                            qT_sb[tp:tp + 64, pair * N + iblk * 512: pair * N + (iblk + 1) * 512],
                            start=True, stop=True, tile_position=(tp, 0))
                    return P

                for j in range(NJ):
                    qkq.append(((iblk, pair, j), lambda j=j: qk(j)))
                while len(Pready) < 2 and qkq:
                    issue_qk()
                for j in range(NJ):
                    if sorted_mode and first_blk and j >= NLEADV:
                        drain_group(f"v{j}")
                    S = sexp.tile([128, 1024], BF16, tag="etil")
                    ebias = (mjb2[:, j:j + 1]
                             if (sorted_mode and iblk == 2) else 0.0)
                    nc.scalar.activation(out=S[:], in_=Pready.pop((iblk, pair, j)),
                                         func=EXPFN, scale=1.0 / 32.0,
                                         bias=ebias)
                    if qkq:
                        issue_qk()
                    for hl in range(2):
                        hcore = 2 * pair + hl
                        soff = j * SJ + hcore * SROW
                        rhs = S[:, hl * 512:(hl + 1) * 512]
                        for v in (2 * hl, 2 * hl + 1):
                            if v not in accs:
                                continue
                            # 128-wide stationary read (cols 65+ produce
                            # ignored partitions) keeps FWL enabled
                            nc.tensor.matmul(
                                accs[v][:], stats[v][:, soff:soff + 128], rhs,
                                start=(j == 0), stop=(j == NJ - 1))
                    if not dual:
                        drain(ndrain)

                # ---- epilogue: select + normalize -----------------------
                na = len(active)
                last_pair = iblk == iblk_order[-1] and pair == NPAIR - 1
                asb = {}
                for v in active:
                    t = episb.tile([65, 512], F32, tag=f"asb{v}", name=f"asb{v}")
                    if last_pair:
                        nc.scalar.activation(
                            out=t, in_=accs[v][0:65, :],
                            func=mybir.ActivationFunctionType.Copy)
                    else:
                        nc.vector.tensor_copy(out=t, in_=accs[v][0:65, :])
                    asb[v] = t
                # reciprocal rows: head-group A (v 0/1) and B (v 2/3) in
                # separate partition-base-0 tiles so the 2-row select works
                rinA = episb.tile([2, 512], F32, tag="rinA")
                rinB = episb.tile([2, 512], F32, tag="rinB")
                rtile = {v: ((rinA, rinB)[v // 2], v % 2 if dual else 0)
                         for v in active}
                for v in active:
                    t, r = rtile[v]
                    nc.sync.dma_start(out=t[r:r + 1, :], in_=asb[v][64:65, :])
                nra = 2 if dual else 1
                nc.vector.reciprocal_approx_fast(out=rinA[0:nra, :],
                                                 in_=rinA[0:nra, :])
                nc.vector.reciprocal_approx_fast(out=rinB[0:nra, :],
                                                 in_=rinB[0:nra, :])
                if dual:
                    ib = iblk * 512
                    nc.vector.tensor_mul(out=rinA[:], in0=rinA[:],
                                         in1=mr4[0:2, ib:ib + 512])
                    nc.vector.tensor_mul(out=rinB[:], in0=rinB[:],
                                         in1=mr4[0:2, ib:ib + 512])
                stg2 = epidr.tile([4, 512], F32, tag="stg2")
                for k, v in enumerate(active):
                    t, r = rtile[v]
                    nc.sync.dma_start(out=stg2[k:k + 1, :], in_=t[r:r + 1, :])
                rball = rblp.tile([64, 4 * 512], F32, tag="rball")
                nc.sync.dma_start(
                    out=rball[:, 0:na * 512],
                    in_=bass.AP(tensor=stg2.tensor, offset=stg2.offset,
                                ap=[[0, 64], [512, na], [1, 512]]))
                rb = {v: rball[:, k * 512:(k + 1) * 512]
                      for k, v in enumerate(active)}
                for hl in range(2):
                    dstc = pair * N + iblk * 512
                    v1, v0 = 2 * hl, 2 * hl + 1
                    if dual:
                        t1 = episb.tile([64, 512], F32, tag="ept1")
                        t2 = episb.tile([64, 512], F32, tag="ept2")
                        nc.vector.tensor_mul(out=t1, in0=asb[v1][0:64, :], in1=rb[v1])
                        nc.vector.tensor_mul(out=t2, in0=asb[v0][0:64, :], in1=rb[v0])
                        if hl == 0:
                            nc.vector.tensor_add(
                                out=attnT[0:64, dstc:dstc + 512], in0=t1, in1=t2)
                        else:
                            t3 = episb.tile([64, 512], BF16, tag="ept3")
                            nc.vector.tensor_add(out=t3, in0=t1, in1=t2)
                            nc.sync.dma_start(
                                out=attnT[64:128, dstc:dstc + 512], in_=t3)
                    else:
                        vv = v1 if v1 in asb else v0
                        if hl == 0:
                            nc.vector.tensor_mul(
                                out=attnT[0:64, dstc:dstc + 512],
                                in0=asb[vv][0:64, :], in1=rb[vv])
                        else:
                            t3 = episb.tile([64, 512], BF16, tag="ept3")
                            nc.vector.tensor_mul(out=t3, in0=asb[vv][0:64, :],
                                                 in1=rb[vv])
                            nc.sync.dma_start(
                                out=attnT[64:128, dstc:dstc + 512], in_=t3)

            if sorted_mode:
                # tags match the free accs of the block where the ops DRAIN:
                # o(0) drains in blk2 {0,2 active}; o(2), o(1) in blk3
                # {1,3 active}; o(3) at the tail.
                tags = ("acc1", "acc3") if iblk == 0 else ("acc0", "acc2")
                pending.extend(
                    o_ops_for_iblk(iblk, tags, fin_act=(iblk == iblk_order[-1])))
            else:
                pending.extend(o_ops_for_iblk(iblk, ("acc0", "acc2"),
                                              fin_act=True))

        # ===== tail: drain remaining ops, keep PE warm =====================
        if sorted_mode:
            warm2 = pacc.tile([128, 512], F32, tag="acc1", name="warm2")
            for _ in range(4):
                nc.tensor.matmul(warm2[:], wo_sb[:, 0:128], wo_sb[:, 0:512],
                                 start=True, stop=True)
        while pending:
            pending.pop(0)[2]()

    if sorted_mode:
        p1sb.release()
    persist.release()
    consts.release()


_CACHE = {}


def build_program(variant="sorted"):
    if variant in _CACHE:
        return _CACHE[variant]
    nc = bacc.Bacc("TRN2", target_bir_lowering=False, debug=False)
    d = {}
    d["xT"] = nc.dram_tensor("xT", (F, N), BF16, kind="ExternalInput").ap()
    d["x8"] = nc.dram_tensor("x8", (F, N), FP8, kind="ExternalInput").ap()
    d["wqp"] = nc.dram_tensor("wqp", (128, 8 * FH), FP8, kind="ExternalInput").ap()
    d["wkp"] = nc.dram_tensor("wkp", (128, 8 * FH), FP8, kind="ExternalInput").ap()
    d["wv"] = nc.dram_tensor("wv", (F, FH), BF16, kind="ExternalInput").ap()
    d["wo"] = nc.dram_tensor("wo", (FH, F), BF16, kind="ExternalInput").ap()
    d["bqk"] = nc.dram_tensor("bqk", (128, 8), F32, kind="ExternalInput").ap()
    d["bvb"] = nc.dram_tensor("bvb", (128, FH), F32, kind="ExternalInput").ap()
    d["ep1"] = nc.dram_tensor("ep1", (128, NTOKC), F32, kind="ExternalInput").ap()
    d["mjb2"] = nc.dram_tensor("mjb2", (128, NJ), F32, kind="ExternalInput").ap()
    d["mr4"] = nc.dram_tensor("mr4", (4, N), F32, kind="ExternalInput").ap()
    d["y"] = nc.dram_tensor("y", (N, F), F32, kind="ExternalOutput").ap()
    with tile.TileContext(nc) as tc:
        _emit(nc, tc, d, sorted_mode=(variant == "sorted"))
    nc.compile()
    _CACHE[variant] = nc
    return nc


def _wpack_qk(w):
    """W [1024, 512] fp32 -> [128, 4*2*512] fp8 x32: [part][g 4][pl 2][col]."""
    a = (w * WS).reshape(4, 2, 128, FH)          # [g, pl, part, col]
    a = a.transpose(2, 0, 1, 3).reshape(128, 8 * FH)
    return a.astype(NPFP8)


def _perm_blocks(m):
    """Permutation putting tokens into blocks: 0 pure-1, 1 mixed, 2 pure
    (mu = c1 > 1024), 3 pure-0. Returns perm, mu."""
    ones = np.flatnonzero(m > 0.5)
    zeros = np.flatnonzero(m <= 0.5)
    c1 = len(ones)
    if c1 > 1024:
        mu = 1.0
        perm = np.concatenate([
            ones[0:512], ones[1024:], zeros[0:1536 - c1],
            ones[512:1024], zeros[1536 - c1:]])
    else:
        mu = 0.0
        perm = np.concatenate([
            ones[0:512], ones[512:], zeros[0:1024 - c1],
            zeros[1024 - c1:1536 - c1], zeros[1536 - c1:]])
    return perm, mu


def make_in_maps(x, inputs_mask, Wq, bq, Wk, bk, Wv, bv, Wo, bo,
                 sorted_mode=True):
    in_maps = []
    m_all = inputs_mask.astype(np.float32)
    perms, mus = [], []
    for b in range(B):
        if sorted_mode:
            p, mu = _perm_blocks(m_all[b])
        else:
            p, mu = np.arange(N), 0.0
        perms.append(p)
        mus.append(mu)
    for c in range(NC_):
        b, hh = c // 2, c % 2
        cs = slice(hh * FH, (hh + 1) * FH)
        m = m_all[b][perms[b]]
        xb = x[b][perms[b]]
        xT = np.ascontiguousarray(xb.T)
        im = {
            "xT": xT.astype(NPBF16),
            "x8": xT.astype(NPFP8),
            "wqp": _wpack_qk(Wq[:, cs]),
            "wkp": _wpack_qk(Wk[:, cs]),
            "wv": Wv[:, cs].astype(NPBF16),
            "wo": np.ascontiguousarray(Wo[cs, :]).astype(NPBF16),
            "bqk": np.stack(
                [bq[cs].reshape(4, 128), bk[cs].reshape(4, 128)], axis=0
            ).reshape(8, 128).T.astype(np.float32).copy(),
            "bvb": np.broadcast_to(bv[cs], (128, FH)).astype(np.float32).copy(),
            "ep1": np.exp(m).reshape(NTOKC, 128).T.astype(np.float32).copy(),
            "mjb2": (mus[b] * m).reshape(NJ, 128).T.astype(np.float32).copy(),
            "mr4": np.stack([m, 1.0 - m, m, 1.0 - m]).astype(np.float32).copy(),
        }
        in_maps.append(im)
    return in_maps, perms


def kernel(x, inputs_mask, Wq, bq, Wk, bk, Wv, bv, Wo, bo):
    x = np.asarray(x, dtype=np.float32)
    inputs_mask = np.asarray(inputs_mask)
    Wq, bq = np.asarray(Wq, np.float32), np.asarray(bq, np.float32)
    Wk, bk = np.asarray(Wk, np.float32), np.asarray(bk, np.float32)
    Wv, bv = np.asarray(Wv, np.float32), np.asarray(bv, np.float32)
    Wo, bo = np.asarray(Wo, np.float32), np.asarray(bo, np.float32)

    c1 = inputs_mask.astype(np.int64).sum(axis=1)
    sorted_mode = bool(np.all((c1 >= 512) & (c1 <= 3 * 512)))
    nc = build_program("sorted" if sorted_mode else "dual")
    in_maps, perms = make_in_maps(
        x, inputs_mask, Wq, bq, Wk, bk, Wv, bv, Wo, bo, sorted_mode=sorted_mode)
    res = bass_utils.run_bass_kernel_spmd(nc, in_maps, core_ids=list(range(NC_)))
    out = np.empty((B, N, F), dtype=np.float32)
    for b in range(B):
        out[b][perms[b]] = (
            res.results[2 * b]["y"] + res.results[2 * b + 1]["y"] + bo
        )
    return out


# revision 25
# speedup vs baseline: 1.0274x; 1.0274x over previous
"""Multi-head self-attention (b=4, n=2048, f=1024, h=16) on 8 trn2 NeuronCores.

Sharding: core c -> batch c//2, head-half c%2 (8 heads of 64 dims each).
Each core computes its 8 heads' attention and a partial output projection
(attn_slice @ Wo_rows); host sums the two partials per batch and adds bo.

v3 over the original kernel (all matmul operands bf16, PSUM fp32):
  - exp has NO bias: the additive mask term exp(m_j) is folded into the AV
    stationaries instead (s1 = e^{+m}[v|1], s0 = [v|1]). This frees the
    choice of AV stationary per query block.
  - the host permutation places tokens so query blocks 0/2/3 are pure
    (block 2's mask value mu arrives as data via a third stationary
    s2 = e^{mu m}[v|1]); only block 1 needs the dual-variant select.
    AV work: 10 matmuls per (pair, key-chunk) vs 12 before.
  - the exp stream on ScalarE (256 x [128,1024] chunks at ~1.15us each,
    ~295us total) is the critical path; PE work (~310us) runs just under
    it. Phase 1 is interleaved into the attention stream: only kT/qT
    chunk 0 and half of v run up front (~30us); the rest of v, kT/qT
    chunks 1-3, and the O-projection drain as micro-ops into the PE slack
    of the pure query blocks, so ScalarE starts exp'ing as early as
    possible and never waits.
"""

import sys

sys.path.insert(0, "/opt/trn_rl_repo")

import numpy as np
import ml_dtypes

import concourse.bass as bass
import concourse.bacc as bacc
import concourse.mybir as mybir
import concourse.tile as tile
from concourse import bass_utils

BF16 = mybir.dt.bfloat16
F32 = mybir.dt.float32
FP8 = mybir.dt.float8e4
NPBF16 = ml_dtypes.bfloat16
NPFP8 = ml_dtypes.float8_e4m3
DR = mybir.MatmulPerfMode.DoubleRow
MUL = mybir.AluOpType.mult
ADD = mybir.AluOpType.add

B, N, F, H, HD = 4, 2048, 1024, 16, 64
FH = 512          # features per core (8 heads)
NC_ = 8           # cores
NTOKC = N // 128  # 16 token chunks
NIBLK = N // 512  # 4 query blocks
NJ = N // 128     # 16 key chunks
NPAIR = 4         # head pairs per core
EXPFN = mybir.ActivationFunctionType.Exp
SROW = 66         # stationary cols per head
SJ = 8 * SROW     # stationary cols per key chunk (528)
NLEADV = 4        # v token-chunks computed up front; the rest drain
WS = 32.0         # q/k weight pre-scale (escapes fp8 subnormals)


def _ap3(t, off, s1, n1, s2, n2):
    """3D AP view [128, n1 (stride s1), n2 (stride s2)] at t+off."""
    return bass.AP(tensor=t.tensor, offset=t.offset + off,
                   ap=[t.ap[0], [s1, n1], [s2, n2]])


def _emit(nc, tc, d, sorted_mode):
    consts = tc.alloc_tile_pool(name="consts", bufs=1)
    persist = tc.alloc_tile_pool(name="persist", bufs=1)

    # ---- persistent activations ----------------------------------------
    qT_sb = persist.tile([128, 4 * N], BF16)   # [feat, tok], fhc at cols fhc*N
    kT_sb = persist.tile([128, 4 * N], BF16)
    s0_sb = persist.tile([128, NJ * SJ + 64], BF16)  # [j][h][66]: [vb|1]
    s1_sb = persist.tile([128, NJ * SJ + 64], BF16)  # e^{+m} * [vb|1]
    attnT = persist.tile([128, 4 * N], BF16)   # normalized attn, [feat, tok]

    # ================= phase 1 (lead-in part) ============================
    p1sb = tc.alloc_tile_pool(name="p1sb", bufs=1)
    pkt = tc.alloc_tile_pool(name="pkt", bufs=1, space="PSUM")

    bqk = consts.tile([128, 8], F32)       # bq chunks (0-3), bk (4-7)
    nc.sync.dma_start(out=bqk, in_=d["bqk"])
    nc.vector.memset(s0_sb[:], 0.0)
    nc.vector.memset(s1_sb[:], 0.0)
    # exp table warm-up (~2.7us) while DMAs run
    warm = consts.tile([128, 8], BF16)
    nc.scalar.activation(out=warm, in_=bqk, func=EXPFN, scale=0.0)

    x8_sb = p1sb.tile([128, 8 * N], FP8)
    wk_sb = p1sb.tile([128, 8 * FH], FP8)  # [g 4][pl 2][col 512], x32
    wq_sb = p1sb.tile([128, 8 * FH], FP8)
    nc.sync.dma_start(out=wk_sb, in_=d["wkp"])
    nc.sync.dma_start(out=wq_sb, in_=d["wqp"])
    for fc in range(8):
        nc.sync.dma_start(out=x8_sb[:, fc * N:(fc + 1) * N],
                          in_=d["x8"][fc * 128:(fc + 1) * 128, :])

    pools = {"proj": pkt}  # phase 2 rebinds this to its own PSUM pool

    def proj_qk_ops(w_sb, fhc, win, bias_col, out_sb, tagsel, grp=None):
        """Micro-ops for one [128,512] window of a q/k DR-fp8 projection:
        4 paired-contraction matmuls plus the scale+bias add."""
        st = {}
        ops = []
        grp = grp or f"f{fhc}"
        for g in range(4):
            def mm(g=g, st=st):
                if g == 0:
                    st["pk"] = pools["proj"].tile([128, 512], F32,
                                                  tag=tagsel, name="pk")
                lhsT = _ap3(w_sb, g * 1024 + fhc * 128, 512, 2, 1, 128)
                rhs = _ap3(x8_sb, (2 * g) * N + win * 512, N, 2, 1, 512)
                nc.tensor.matmul(st["pk"][:], lhsT, rhs, start=(g == 0),
                                 stop=(g == 3), perf_mode=DR)
            ops.append(("pe", grp, mm))

        def fin(st=st):
            nc.vector.tensor_scalar(
                out=out_sb[:, fhc * N + win * 512: fhc * N + win * 512 + 512],
                in0=st["pk"][:], scalar1=1.0 / WS,
                scalar2=bqk[:, bias_col:bias_col + 1], op0=MUL, op1=ADD)
        ops.append(("dve", grp, fin))
        return ops

    # up front: all of kT chunk 0, and qT chunk 0's first-block window
    if sorted_mode:
        for win in range(4):
            for _, _, op in proj_qk_ops(wk_sb, 0, win, 4, kT_sb,
                                        f"pp{win % 2}"):
                op()
        for _, _, op in proj_qk_ops(wq_sb, 0, 0, 0, qT_sb, "pp2"):
            op()
    else:
        for fhc in range(4):
            for win in range(4):
                for _, _, op in proj_qk_ops(wk_sb, fhc, win, 4 + fhc, kT_sb,
                                            f"pp{win % 2}"):
                    op()
                for _, _, op in proj_qk_ops(wq_sb, fhc, win, fhc, qT_sb,
                                            f"pp{2 + win % 2}"):
                    op()

    # loads for the v/attention part; x (bf16) arrives token-block-major so
    # v(tokc) starts after 1/4 of it
    xT_sb = p1sb.tile([128, 8 * N], BF16)
    wv_sb = p1sb.tile([128, 8 * FH], BF16)
    for fc in range(8):
        nc.sync.dma_start(out=wv_sb[:, fc * FH:(fc + 1) * FH],
                          in_=d["wv"][fc * 128:(fc + 1) * 128, :])
    for tb in range(4):
        for fc in range(8):
            nc.sync.dma_start(
                out=xT_sb[:, fc * N + tb * 512: fc * N + (tb + 1) * 512],
                in_=d["xT"][fc * 128:(fc + 1) * 128, tb * 512:(tb + 1) * 512])
    ep1 = consts.tile([128, NTOKC], F32)   # e^{+m}
    nc.sync.dma_start(out=ep1, in_=d["ep1"])
    mjb2 = consts.tile([128, NJ], F32)     # exp bias for block 2: mu*m_j
    nc.sync.dma_start(out=mjb2, in_=d["mjb2"])
    bvb = consts.tile([128, FH], F32)
    nc.sync.dma_start(out=bvb, in_=d["bvb"])
    nmr = 2 if sorted_mode else 4
    mr4 = consts.tile([nmr, N], F32)       # select rows [m, 1-m, (m, 1-m)]
    nc.sync.dma_start(out=mr4, in_=d["mr4"][0:nmr, :])
    wo_sb = consts.tile([128, 4 * 1024], BF16)
    for fc in range(4):
        nc.sync.dma_start(out=wo_sb[:, fc * 1024:(fc + 1) * 1024],
                          in_=d["wo"][fc * 128:(fc + 1) * 128, :])

    # v projection + stationaries; AV consumes s[j] at key chunk j
    vstg = p1sb.tile([128, FH], F32, tag="vstg")

    def v_ops(tokc, tagsel):
        st = {}
        ops = []
        grp = f"v{tokc}"
        for fc in range(8):
            def mm(fc=fc, tokc=tokc, st=st):
                if fc == 0:
                    st["pv"] = pools["proj"].tile([128, FH], F32,
                                                  tag=tagsel, name="pv")
                nc.tensor.matmul(
                    st["pv"][:],
                    xT_sb[:, fc * N + tokc * 128: fc * N + (tokc + 1) * 128],
                    wv_sb[:, fc * FH:(fc + 1) * FH],
                    start=(fc == 0), stop=(fc == 7))
            ops.append(("pe", grp, mm))

        def fin(tokc=tokc, st=st):
            nc.vector.tensor_add(out=vstg, in0=st["pv"][:], in1=bvb)
            base = tokc * SJ
            vv = vstg[:].rearrange("p (h c) -> p h c", h=8)
            for s_sb, scol in ((s0_sb, None), (s1_sb, ep1)):
                sv = s_sb[:, base:base + SJ].rearrange("p (h c) -> p h c", h=8)
                if scol is None:
                    nc.vector.tensor_copy(out=sv[:, :, 0:64], in_=vv)
                    nc.vector.memset(sv[:, :, 64:65], 1.0)
                else:
                    nc.vector.tensor_scalar_mul(
                        out=sv[:, :, 0:64], in0=vv,
                        scalar1=scol[:, tokc:tokc + 1])
                    colb = bass.AP(
                        tensor=scol.tensor,
                        offset=scol[:, tokc:tokc + 1].offset,
                        ap=[scol[:, tokc:tokc + 1].ap[0], [0, 8], [1, 1]])
                    nc.vector.tensor_copy(out=sv[:, :, 64:65], in_=colb)
        ops.append(("dve", grp, fin))
        return ops

    nleadv = NLEADV if sorted_mode else NTOKC
    for tokc in range(nleadv):
        for _, _, op in v_ops(tokc, f"pp{tokc % 2}"):
            op()

    # deferred work drains into attention slack (pure blocks)
    pending = []
    if sorted_mode:
        pending.extend(proj_qk_ops(wq_sb, 0, 2, 0, qT_sb, "acc3", grp="f0w2"))
        for tokc in range(NLEADV, 8):
            pending.extend(v_ops(tokc, "acc1" if tokc % 2 else "acc3"))
        for win in range(4):
            pending.extend(
                proj_qk_ops(wk_sb, 1, win, 5, kT_sb, "acc1"))
            pending.extend(
                proj_qk_ops(wq_sb, 1, win, 1, qT_sb, "acc3"))
        for tokc in range(8, NTOKC):
            pending.extend(v_ops(tokc, "acc1" if tokc % 2 else "acc3"))
        for fhc in (2, 3):
            for win in range(4):
                pending.extend(
                    proj_qk_ops(wk_sb, fhc, win, 4 + fhc, kT_sb, "acc1"))
                pending.extend(
                    proj_qk_ops(wq_sb, fhc, win, fhc, qT_sb, "acc3"))
        pending.extend(proj_qk_ops(wq_sb, 0, 1, 0, qT_sb, "acc3", grp="f0w1"))
        pending.extend(proj_qk_ops(wq_sb, 0, 3, 0, qT_sb, "acc3", grp="f0w3"))

    pkt.release()
    if not sorted_mode:
        p1sb.release()  # nothing defers in dual mode; free xT/w space

    def drain(npe):
        done = 0
        while pending and done < npe:
            kind, _, op = pending.pop(0)
            op()
            if kind == "pe":
                done += 1
        # trailing dve ops ride along for free
        while pending and pending[0][0] == "dve":
            pending.pop(0)[2]()

    def drain_group(grp):
        """Pop (in order) until no ops of group `grp` remain."""
        while any(g == grp for _, g, _ in pending):
            pending.pop(0)[2]()

    # ================= phase 2: attention ================================
    with tc.tile_pool(name="pP", bufs=2, space="PSUM") as pP, \
         tc.tile_pool(name="pacc", bufs=1, space="PSUM") as pacc, \
         tc.tile_pool(name="sexp", bufs=3) as sexp, \
         tc.tile_pool(name="episb", bufs=1) as episb, \
         tc.tile_pool(name="rblp", bufs=1) as rblp, \
         tc.tile_pool(name="osb", bufs=2) as osb, \
         tc.tile_pool(name="epidr", bufs=2, space="DRAM") as epidr:

        pools["proj"] = pacc

        def o_ops_for_iblk(ib, tags, fin_act=False):
            ops = []
            from itertools import cycle
            tagc = cycle(tags)
            for tokc in range(ib * 4, ib * 4 + 4):
                for half in range(2):
                    st = {}
                    for fc in range(4):
                        def mm(fc=fc, tokc=tokc, half=half, st=st):
                            if fc == 0:
                                st["po"] = pacc.tile(
                                    [128, 512], F32, tag=next(tagc), name="po")
                            nc.tensor.matmul(
                                st["po"][:],
                                attnT[:, fc * N + tokc * 128: fc * N + (tokc + 1) * 128],
                                wo_sb[:, fc * 1024 + half * 512: fc * 1024 + half * 512 + 512],
                                start=(fc == 0), stop=(fc == 3))
                        ops.append(("pe", f"o{ib}", mm))

                    def fin(tokc=tokc, half=half, st=st):
                        ot = osb.tile([128, 512], F32, tag="ot", name="ot")
                        if fin_act:
                            nc.scalar.activation(
                                out=ot, in_=st["po"][:],
                                func=mybir.ActivationFunctionType.Copy)
                        else:
                            nc.vector.tensor_copy(out=ot, in_=st["po"][:])
                        nc.sync.dma_start(
                            out=d["y"][tokc * 128:(tokc + 1) * 128,
                                       half * 512:(half + 1) * 512],
                            in_=ot)
                    ops.append(("dve", f"o{ib}", fin))
            return ops

        # global QK issue stream: stays 2 exp-steps ahead ACROSS pair and
        # block boundaries so the exp stream never sees a pipeline refill
        qkq = []          # [(key, closure)] not yet issued
        Pready = {}       # key -> psum logits tile

        def issue_qk():
            key, fn = qkq.pop(0)
            Pready[key] = fn()

        # pure blocks first for drain slack; the dual block third
        iblk_order = [0, 2, 1, 3] if sorted_mode else [0, 1, 2, 3]
        for iblk in iblk_order:
            if sorted_mode and iblk == 0:
                active, stats = [0, 2], {0: s1_sb, 2: s1_sb}
            elif sorted_mode and iblk == 2:
                active, stats = [0, 2], {0: s0_sb, 2: s0_sb}
            elif sorted_mode and iblk == 3:
                active, stats = [1, 3], {1: s0_sb, 3: s0_sb}
            else:
                active = [0, 1, 2, 3]
                stats = {0: s1_sb, 1: s0_sb, 2: s1_sb, 3: s0_sb}
            dual = len(active) == 4
            ndrain = 6 if (sorted_mode and iblk == 0) else 2
            first_blk = iblk == iblk_order[0]
            for pair in range(NPAIR):
                accs = {
                    v: pacc.tile([128, 512], F32, tag=f"acc{v}", name=f"acc{v}")
                    for v in active
                }
                def qk(j, pair=pair, iblk=iblk):
                    # deferred kT/qT chunks must be emitted before this
                    # pair's first QK reads them
                    if sorted_mode and j == 0 and pair > 0:
                        drain_group(f"f{pair}")
                    if sorted_mode and j == 0 and pair == 0 and iblk != 0:
                        drain_group(f"f0w{iblk}")
                    P = pP.tile([128, 1024], F32, tag="logits")
                    for hl, tp in ((0, 0), (1, 64)):
                        nc.tensor.matmul(
                            P[:, hl * 512:(hl + 1) * 512],
                            kT_sb[tp:tp + 64, pair * N + j * 128: pair * N + (j + 1) * 128],
                            qT_sb[tp:tp + 64, pair * N + iblk * 512: pair * N + (iblk + 1) * 512],
                            start=True, stop=True, tile_position=(tp, 0))
                    return P

                for j in range(NJ):
                    qkq.append(((iblk, pair, j), lambda j=j: qk(j)))
                while len(Pready) < 2 and qkq:
                    issue_qk()
                for j in range(NJ):
                    if sorted_mode and first_blk and j >= NLEADV:
                        drain_group(f"v{j}")
                    S = sexp.tile([128, 1024], BF16, tag="etil")
                    ebias = (mjb2[:, j:j + 1]
                             if (sorted_mode and iblk == 2) else 0.0)
                    nc.scalar.activation(out=S[:], in_=Pready.pop((iblk, pair, j)),
                                         func=EXPFN, scale=1.0 / 32.0,
                                         bias=ebias)
                    if qkq:
                        issue_qk()
                    for hl in range(2):
                        hcore = 2 * pair + hl
                        soff = j * SJ + hcore * SROW
                        rhs = S[:, hl * 512:(hl + 1) * 512]
                        for v in (2 * hl, 2 * hl + 1):
                            if v not in accs:
                                continue
                            # 128-wide stationary read (cols 65+ produce
                            # ignored partitions) keeps FWL enabled
                            nc.tensor.matmul(
                                accs[v][:], stats[v][:, soff:soff + 128], rhs,
                                start=(j == 0), stop=(j == NJ - 1))
                    if not dual:
                        drain(ndrain)

                # ---- epilogue: select + normalize -----------------------
                na = len(active)
                last_pair = iblk == iblk_order[-1] and pair == NPAIR - 1
                asb = {}
                for v in active:
                    t = episb.tile([65, 512], F32, tag=f"asb{v}", name=f"asb{v}")
                    if last_pair:
                        nc.scalar.activation(
                            out=t, in_=accs[v][0:65, :],
                            func=mybir.ActivationFunctionType.Copy)
                    else:
                        nc.vector.tensor_copy(out=t, in_=accs[v][0:65, :])
                    asb[v] = t
                # reciprocal rows: head-group A (v 0/1) and B (v 2/3) in
                # separate partition-base-0 tiles so the 2-row select works
                rinA = episb.tile([2, 512], F32, tag="rinA")
                rinB = episb.tile([2, 512], F32, tag="rinB")
                rtile = {v: ((rinA, rinB)[v // 2], v % 2 if dual else 0)
                         for v in active}
                for v in active:
                    t, r = rtile[v]
                    nc.sync.dma_start(out=t[r:r + 1, :], in_=asb[v][64:65, :])
                nra = 2 if dual else 1
                nc.vector.reciprocal_approx_fast(out=rinA[0:nra, :],
                                                 in_=rinA[0:nra, :])
                nc.vector.reciprocal_approx_fast(out=rinB[0:nra, :],
                                                 in_=rinB[0:nra, :])
                if dual:
                    ib = iblk * 512
                    nc.vector.tensor_mul(out=rinA[:], in0=rinA[:],
                                         in1=mr4[0:2, ib:ib + 512])
                    nc.vector.tensor_mul(out=rinB[:], in0=rinB[:],
                                         in1=mr4[0:2, ib:ib + 512])
                stg2 = epidr.tile([4, 512], F32, tag="stg2")
                for k, v in enumerate(active):
                    t, r = rtile[v]
                    nc.sync.dma_start(out=stg2[k:k + 1, :], in_=t[r:r + 1, :])
                rball = rblp.tile([64, 4 * 512], F32, tag="rball")
                nc.sync.dma_start(
                    out=rball[:, 0:na * 512],
                    in_=bass.AP(tensor=stg2.tensor, offset=stg2.offset,
                                ap=[[0, 64], [512, na], [1, 512]]))
                rb = {v: rball[:, k * 512:(k + 1) * 512]
                      for k, v in enumerate(active)}
                for hl in range(2):
                    dstc = pair * N + iblk * 512
                    v1, v0 = 2 * hl, 2 * hl + 1
                    if dual:
                        t1 = episb.tile([64, 512], F32, tag="ept1")
                        t2 = episb.tile([64, 512], F32, tag="ept2")
                        nc.vector.tensor_mul(out=t1, in0=asb[v1][0:64, :], in1=rb[v1])
                        nc.vector.tensor_mul(out=t2, in0=asb[v0][0:64, :], in1=rb[v0])
                        if hl == 0:
                            nc.vector.tensor_add(
                                out=attnT[0:64, dstc:dstc + 512], in0=t1, in1=t2)
                        else:
                            t3 = episb.tile([64, 512], BF16, tag="ept3")
                            nc.vector.tensor_add(out=t3, in0=t1, in1=t2)
                            nc.sync.dma_start(
                                out=attnT[64:128, dstc:dstc + 512], in_=t3)
                    else:
                        vv = v1 if v1 in asb else v0
                        if hl == 0:
                            nc.vector.tensor_mul(
                                out=attnT[0:64, dstc:dstc + 512],
                                in0=asb[vv][0:64, :], in1=rb[vv])
                        else:
                            t3 = episb.tile([64, 512], BF16, tag="ept3")
                            nc.vector.tensor_mul(out=t3, in0=asb[vv][0:64, :],
                                                 in1=rb[vv])
                            nc.sync.dma_start(
                                out=attnT[64:128, dstc:dstc + 512], in_=t3)

            if sorted_mode:
                # tags match the free accs of the block where the ops DRAIN:
                # o(0) drains in blk2 {0,2 active}; o(2), o(1) in blk3
                # {1,3 active}; o(3) at the tail.
                tags = ("acc1", "acc3") if iblk == 0 else ("acc0", "acc2")
                pending.extend(
                    o_ops_for_iblk(iblk, tags, fin_act=(iblk == iblk_order[-1])))
            else:
                pending.extend(o_ops_for_iblk(iblk, ("acc0", "acc2"),
                                              fin_act=True))

        # ===== tail: drain remaining ops, keep PE warm =====================
        if sorted_mode:
            warm2 = pacc.tile([128, 512], F32, tag="acc1", name="warm2")
            for _ in range(4):
                nc.tensor.matmul(warm2[:], wo_sb[:, 0:128], wo_sb[:, 0:512],
                                 start=True, stop=True)
        while pending:
            pending.pop(0)[2]()

    if sorted_mode:
        p1sb.release()
    persist.release()
    consts.release()


_CACHE = {}


def build_program(variant="sorted"):
    if variant in _CACHE:
        return _CACHE[variant]
    nc = bacc.Bacc("TRN2", target_bir_lowering=False, debug=False)
    d = {}
    d["xT"] = nc.dram_tensor("xT", (F, N), BF16, kind="ExternalInput").ap()
    d["x8"] = nc.dram_tensor("x8", (F, N), FP8, kind="ExternalInput").ap()
    d["wqp"] = nc.dram_tensor("wqp", (128, 8 * FH), FP8, kind="ExternalInput").ap()
    d["wkp"] = nc.dram_tensor("wkp", (128, 8 * FH), FP8, kind="ExternalInput").ap()
    d["wv"] = nc.dram_tensor("wv", (F, FH), BF16, kind="ExternalInput").ap()
    d["wo"] = nc.dram_tensor("wo", (FH, F), BF16, kind="ExternalInput").ap()
    d["bqk"] = nc.dram_tensor("bqk", (128, 8), F32, kind="ExternalInput").ap()
    d["bvb"] = nc.dram_tensor("bvb", (128, FH), F32, kind="ExternalInput").ap()
    d["ep1"] = nc.dram_tensor("ep1", (128, NTOKC), F32, kind="ExternalInput").ap()
    d["mjb2"] = nc.dram_tensor("mjb2", (128, NJ), F32, kind="ExternalInput").ap()
    d["mr4"] = nc.dram_tensor("mr4", (4, N), F32, kind="ExternalInput").ap()
    d["y"] = nc.dram_tensor("y", (N, F), F32, kind="ExternalOutput").ap()
    with tile.TileContext(nc) as tc:
        _emit(nc, tc, d, sorted_mode=(variant == "sorted"))
    nc.compile()
    _CACHE[variant] = nc
    return nc


def _wpack_qk(w):
    """W [1024, 512] fp32 -> [128, 4*2*512] fp8 x32: [part][g 4][pl 2][col]."""
    a = (w * WS).reshape(4, 2, 128, FH)          # [g, pl, part, col]
    a = a.transpose(2, 0, 1, 3).reshape(128, 8 * FH)
    return a.astype(NPFP8)


def _perm_blocks(m):
    """Permutation putting tokens into blocks: 0 pure-1, 1 mixed, 2 pure
    (mu = c1 > 1024), 3 pure-0. Returns perm, mu."""
    ones = np.flatnonzero(m > 0.5)
    zeros = np.flatnonzero(m <= 0.5)
    c1 = len(ones)
    if c1 > 1024:
        mu = 1.0
        perm = np.concatenate([
            ones[0:512], ones[1024:], zeros[0:1536 - c1],
            ones[512:1024], zeros[1536 - c1:]])
    else:
        mu = 0.0
        perm = np.concatenate([
            ones[0:512], ones[512:], zeros[0:1024 - c1],
            zeros[1024 - c1:1536 - c1], zeros[1536 - c1:]])
    return perm, mu


def make_in_maps(x, inputs_mask, Wq, bq, Wk, bk, Wv, bv, Wo, bo,
                 sorted_mode=True):
    in_maps = []
    m_all = inputs_mask.astype(np.float32)
    perms, mus = [], []
    for b in range(B):
        if sorted_mode:
            p, mu = _perm_blocks(m_all[b])
        else:
            p, mu = np.arange(N), 0.0
        perms.append(p)
        mus.append(mu)
    for c in range(NC_):
        b, hh = c // 2, c % 2
        cs = slice(hh * FH, (hh + 1) * FH)
        m = m_all[b][perms[b]]
        xb = x[b][perms[b]]
        xT = np.ascontiguousarray(xb.T)
        im = {
            "xT": xT.astype(NPBF16),
            "x8": xT.astype(NPFP8),
            "wqp": _wpack_qk(Wq[:, cs]),
            "wkp": _wpack_qk(Wk[:, cs]),
            "wv": Wv[:, cs].astype(NPBF16),
            "wo": np.ascontiguousarray(Wo[cs, :]).astype(NPBF16),
            "bqk": np.stack(
                [bq[cs].reshape(4, 128), bk[cs].reshape(4, 128)], axis=0
            ).reshape(8, 128).T.astype(np.float32).copy(),
            "bvb": np.broadcast_to(bv[cs], (128, FH)).astype(np.float32).copy(),
            "ep1": np.exp(m).reshape(NTOKC, 128).T.astype(np.float32).copy(),
            "mjb2": (mus[b] * m).reshape(NJ, 128).T.astype(np.float32).copy(),
            "mr4": np.stack([m, 1.0 - m, m, 1.0 - m]).astype(np.float32).copy(),
        }
        in_maps.append(im)
    return in_maps, perms


def kernel(x, inputs_mask, Wq, bq, Wk, bk, Wv, bv, Wo, bo):
    x = np.asarray(x, dtype=np.float32)
    inputs_mask = np.asarray(inputs_mask)
    Wq, bq = np.asarray(Wq, np.float32), np.asarray(bq, np.float32)
    Wk, bk = np.asarray(Wk, np.float32), np.asarray(bk, np.float32)
    Wv, bv = np.asarray(Wv, np.float32), np.asarray(bv, np.float32)
    Wo, bo = np.asarray(Wo, np.float32), np.asarray(bo, np.float32)

    c1 = inputs_mask.astype(np.int64).sum(axis=1)
    sorted_mode = bool(np.all((c1 >= 512) & (c1 <= 3 * 512)))
    nc = build_program("sorted" if sorted_mode else "dual")
    in_maps, perms = make_in_maps(
        x, inputs_mask, Wq, bq, Wk, bk, Wv, bv, Wo, bo, sorted_mode=sorted_mode)
    res = bass_utils.run_bass_kernel_spmd(nc, in_maps, core_ids=list(range(NC_)))
    out = np.empty((B, N, F), dtype=np.float32)
    for b in range(B):
        out[b][perms[b]] = (
            res.results[2 * b]["y"] + res.results[2 * b + 1]["y"] + bo
        )
    return out
